# revision 25
# baseline (speedup 1.0000x reference)
"""Multi-head attention (B=2, S=2048, D=1024, H=16) on 8 trn2 NeuronCores.

Sharding: 2-way batch x 4-way head-group tensor parallel. Core c handles
batch c//4 and heads 4*(c%4) .. 4*(c%4)+3 (a 256-wide feature slice of the
q/k/v projections, and the matching row-slice of the out projection). Each
core emits a full-size [2048, 1024] partial of the output; the host sums the
4 partials per batch and adds the output bias.

Everything on the wire and in SBUF is bf16 (half the HBM traffic, FWL
weight loads; mixing f32r and bf16 matmuls in one kernel measured slower
than uniform bf16). PSUM accumulation is always f32; the host sums the
bf16 partials in f32.

On-device dataflow (per core):
  phase A: x arrives feature-major bf16; first K chunk is DMA'd in per-ft
           slices so the first projection matmul can start as soon as
           ~128KB has landed instead of waiting for the full prefetch
           window. Project to QT/KT [dq, t] (feature-major) and V
           [t, dv] (token-major). V gets a 64-wide block of ones
           columns appended so the attn.V matmul also produces the softmax
           denominator replicated on psum partitions 64..127.
  phase B: per (q-chunk, head): scoresT[k, q] = KT_h.T @ QT_h on PE.
           exp(0.125*s) is split per psum-buffer parity: even k-groups on
           ScalarE (table exp), odd k-groups on VectorE via a Schraudolph
           bit-trick (t = s*(SCALE*2^7*log2e) + magic, converted to int16
           and bit-viewed as bf16 ~= exp with +-4% ripple; softmax
           renormalization uses the same approximated weights, and the
           final tolerance is 2e-2). Then outT'[128, q] = sum_k V''_h.T @ P;
           rows 64..127 are the denominator; normalize rows 0..63 via
           reciprocal_approx_fast + multiply on DVE (the denominator is
           staged through a partition-0 SBUF tile first: custom-DVE ops
           misread PSUM slices at base_partition 64 on HW). The chunk's
           out-projection matmuls follow, with PSUM->SBUF staging copies on
           ScalarE (keeps DVE free for the Schraudolph share).
"""

import ml_dtypes
import numpy as np

import concourse.bacc as bacc
import concourse.bass as bass
import concourse.mybir as mybir
import concourse.tile as tile
from concourse.bass_interp import get_hw_module
from concourse.bass_utils import run_bass_kernel_spmd
from concourse.masks import make_identity

# problem constants (hardcoded; must match the reference)
B = 2
S = 2048
D = 1024
NH = 16
DH = 64
SCALE = DH ** -0.5

# sharding
N_CORES = 8
HG = 4                # heads per core
F = HG * DH           # 256 projected features per core
CH = 512              # token chunk
NCH = S // CH         # 4 chunks
P = 128
FT = D // P           # 8 feature tiles
MT = F // P           # 2 projected-feature tiles
KT = S // P           # 16 key-token tiles

f32 = mybir.dt.float32
f32r = mybir.dt.float32r
i32 = mybir.dt.int32
i16 = mybir.dt.int16
bf16 = mybir.dt.bfloat16
EXP = mybir.ActivationFunctionType.Exp
BF16 = ml_dtypes.bfloat16

# Schraudolph exp-in-bf16-bits constants: round(x*2^7*log2e + (127<<7) - C)
# viewed as bf16 ~= exp(x), C = 0.0579*2^7 centers the relative error at
# ~+-3%. SCALE is folded into the multiplier. (bf16 rather than f32r because
# the BIR verifier requires f32r-matmul producers to round to f32r, which a
# bit-pattern write can't satisfy; bf16 has no such constraint.)
SCHR_A = SCALE * (2 ** 7) * 1.4426950408889634
SCHR_B = float((127 << 7) - 7.41)


def _emit(ctx, nc, tc, aps):
    xqT, xkT, xvT, wqT, wkT, wvT, woT, bq2, bk2, bv1, out = aps

    consts = ctx.enter_context(tc.tile_pool(name="consts", bufs=1))
    persist = ctx.enter_context(tc.tile_pool(name="persist", bufs=1))
    bq_sb = consts.tile([P, MT], f32)
    bk_sb = consts.tile([P, MT], f32)
    bv_sb = consts.tile([P, F], f32)
    wo_sb = consts.tile([P, MT, D], bf16)
    nc.scalar.dma_start(out=bq_sb, in_=bq2)
    nc.scalar.dma_start(out=bk_sb, in_=bk2)
    nc.scalar.dma_start(out=bv_sb, in_=bv1.unsqueeze(0).to_broadcast((P, F)))

    # persistent activations
    QT_sb = persist.tile([P, MT, NCH, CH], bf16)   # [dq%128, dq//128, qc, q]
    # KT, zero-padded to full-K contraction: variant par holds head parity
    # par's 64 feature rows, zeros in the other 64. A scores matmul then uses
    # a full [128, 128] stationary operand (K=64 descriptors run at half PE
    # rate), with the zeros annihilating the other head's QT rows.
    KTz_sb = persist.tile([P, 2, MT, NCH, CH], bf16)
    # V'' layout: [k%128, k//128, h, dv | 64 ones columns]
    V_sb = persist.tile([P, KT, HG, P], bf16)
    ones_sb = consts.tile([P, 1], f32)
    zeros_sb = consts.tile([P, 1], f32)

    with tc.tile_pool(name="w_pool", bufs=3) as w_pool, \
         tc.tile_pool(name="xT_pool", bufs=4) as xT_pool, \
         tc.tile_pool(name="ps_proj", bufs=4, space="PSUM") as ps_proj:
        # phase A: load feature-major x chunks, project. K and Q first so
        # attention score/exp work can begin while V still projects.
        w_tiles = {}
        # chunk order: K(all) -> Q(chunk 0) -> V(all) -> Q(chunks 1-3).
        # Scores for qc0 only need K + Q(0), so they start as early as
        # before, while V now finishes ~6us earlier and un-gates the
        # first attn.V matmuls; Q chunks 1-3 are not needed until qc=1.
        sched = [(0, (0, 1, 2, 3)), (1, (0,)), (2, (0, 1, 2, 3)),
                 (1, (1, 2, 3))]
        for which, chunks in sched:
            xT_ap, wT_ap = ((xkT, wkT), (xqT, wqT), (xvT, wvT))[which]
            if which in w_tiles:
                w_sb = w_tiles[which]
            else:
                w_sb = w_pool.tile([P, FT, F], bf16, tag=f"w{which}")
                w_tiles[which] = w_sb
            if which == 0:
                # split the critical first weight load so low ft tiles land
                # ahead of the round-robin prefetch traffic
                for fh in range(2):
                    nc.scalar.dma_start(
                        out=w_sb[:, 4 * fh:4 * fh + 4, :],
                        in_=wT_ap.rearrange("(ft p) m -> p ft m", p=P)[
                            :, 4 * fh:4 * fh + 4, :
                        ],
                    )
            elif chunks[0] == 0:
                nc.scalar.dma_start(
                    out=w_sb, in_=wT_ap.rearrange("(ft p) m -> p ft m", p=P)
                )
            if which == 0:
                nc.vector.memset(ones_sb, 1.0)
                nc.vector.memset(zeros_sb, 0.0)
                nc.vector.tensor_copy(
                    V_sb[:, :, :, DH:P],
                    ones_sb.to_broadcast((P, KT, HG, P - DH)),
                )
                nc.vector.tensor_copy(
                    KTz_sb[DH:P, 0],
                    zeros_sb[DH:P].to_broadcast((DH, MT, NCH, CH)),
                )
                nc.vector.tensor_copy(
                    KTz_sb[0:DH, 1],
                    zeros_sb[0:DH].to_broadcast((DH, MT, NCH, CH)),
                )
            for c in chunks:
                xT = xT_pool.tile([P, FT, CH], bf16, tag="xT")
                xin = xT_ap[:, c * CH:(c + 1) * CH].rearrange(
                    "(ft p) t -> p ft t", p=P
                )
                if which == 0 and c == 0:
                    # per-ft DMAs: the first projection matmuls only need
                    # ft=0, so compute starts ~10us earlier than waiting
                    # for the whole 1MB chunk behind the prefetch
                    # queue round-robin.
                    for fq in range(4):
                        nc.sync.dma_start(
                            out=xT[:, 2 * fq:2 * fq + 2, :],
                            in_=xin[:, 2 * fq:2 * fq + 2, :],
                        )
                else:
                    nc.sync.dma_start(out=xT, in_=xin)
                if which != 2:  # Q / K: feature-major [dq, t]
                    is_q = which == 1
                    b_sb = bq_sb if is_q else bk_sb
                    for m in range(MT):
                        ps = ps_proj.tile([P, CH], f32, tag="proj")
                        for ft in range(FT):
                            nc.tensor.matmul(
                                ps,
                                w_sb[:, ft, m * P:(m + 1) * P],
                                xT[:, ft, :],
                                start=(ft == 0),
                                stop=(ft == FT - 1),
                            )
                        if is_q:
                            nc.vector.tensor_scalar_add(
                                QT_sb[:, m, c, :], ps, b_sb[:, m:m + 1]
                            )
                        else:
                            nc.vector.tensor_scalar_add(
                                KTz_sb[0:DH, 0, m, c, :], ps[0:DH, :],
                                b_sb[0:DH, m:m + 1],
                            )
                            nc.vector.tensor_scalar_add(
                                KTz_sb[DH:P, 1, m, c, :], ps[DH:P, :],
                                b_sb[DH:P, m:m + 1],
                            )
                else:  # V: token-major [t, dv]
                    for t4 in range(CH // P):
                        ps = ps_proj.tile([P, F], f32, tag="proj")
                        for ft in range(FT):
                            nc.tensor.matmul(
                                ps,
                                xT[:, ft, t4 * P:(t4 + 1) * P],
                                w_sb[:, ft, :],
                                start=(ft == 0),
                                stop=(ft == FT - 1),
                            )
                        kt = c * (CH // P) + t4
                        nc.vector.tensor_add(
                            V_sb[:, kt, :, 0:DH],
                            ps.rearrange("p (h d) -> p h d", h=HG),
                            bv_sb.rearrange("p (h d) -> p h d", h=HG),
                        )
            if which == 0:
                # out-proj weights: first needed ~90us in; issue after K's
                # critical activation chunks so they don't dilute the
                # round-robin DMA bandwidth at startup
                nc.scalar.dma_start(
                    out=wo_sb, in_=woT.rearrange("(m p) e -> p m e", p=P)
                )

    with tc.tile_pool(name="ps_s", bufs=2, space="PSUM") as ps_s, \
         tc.tile_pool(name="ps_o", bufs=2, space="PSUM") as ps_o, \
         tc.tile_pool(name="ps_out", bufs=2, space="PSUM") as ps_out, \
         tc.tile_pool(name="pt_pool", bufs=2) as pt_pool, \
         tc.tile_pool(name="ot_pool", bufs=1) as ot_pool, \
         tc.tile_pool(name="o_stage", bufs=3) as o_stage, \
         tc.tile_pool(name="rc_pool", bufs=1) as rc_pool:
        OT_sb = ot_pool.tile([P, MT, NCH, CH], bf16)

        def outproj_t4(pqc, t4):
            ob = o_stage.tile([P, D], bf16, tag="ob")
            tt = pqc * NCH + t4
            for n2 in range(D // CH):
                ps = ps_out.tile([P, CH], f32, tag="po")
                for m in range(MT):
                    nc.tensor.matmul(
                        ps,
                        OT_sb[:, m, pqc, t4 * P:(t4 + 1) * P],
                        wo_sb[:, m, n2 * CH:(n2 + 1) * CH],
                        start=(m == 0),
                        stop=(m == MT - 1),
                    )
                if n2 == 0:
                    nc.scalar.copy(ob[:, n2 * CH:(n2 + 1) * CH], ps)
                else:
                    nc.vector.tensor_copy(ob[:, n2 * CH:(n2 + 1) * CH], ps)
                nc.sync.dma_start(
                    out=out[tt * P:(tt + 1) * P, n2 * CH:(n2 + 1) * CH],
                    in_=ob[:, n2 * CH:(n2 + 1) * CH],
                )

        # phase B: attention per (q-chunk, head); the PREVIOUS chunk's
        # out-projection tiles are interleaved one-per-head so their
        # PSUM->SBUF staging copies never cluster on the ScalarE/DVE FIFOs
        # at a chunk boundary (which would delay the next chunk's exp)
        for qc in range(NCH):
            for h in range(HG):
                mh, p0 = divmod(h, 2)
                p0 *= DH
                PT = pt_pool.tile([P, KT, CH], bf16, tag="PT")
                par = h % 2
                for kg in range(KT // 2):  # 2 k-tiles share a psum group
                    ps = ps_s.tile([P, 2, CH], f32, tag="s")
                    for j in range(2):
                        kt = kg * 2 + j
                        nc.tensor.matmul(
                            ps[:, j, :],
                            KTz_sb[:, par, mh, kt // 4,
                                   (kt % 4) * P:(kt % 4) * P + P],
                            QT_sb[:, mh, qc, :],
                            start=True,
                            stop=True,
                        )
                    if kg % 2 == 0 or kg == 7:
                        nc.scalar.activation(
                            out=PT[:, kg * 2:kg * 2 + 2, :], in_=ps,
                            func=EXP, scale=SCALE,
                        )
                    else:
                        nc.vector.tensor_scalar(
                            out=PT[:, kg * 2:kg * 2 + 2, :].bitcast(i16),
                            in0=ps,
                            scalar1=SCHR_A,
                            scalar2=SCHR_B,
                            op0=mybir.AluOpType.mult,
                            op1=mybir.AluOpType.add,
                        )
                po = ps_o.tile([P, CH], f32, tag="o")
                for kt in range(KT):
                    nc.tensor.matmul(
                        po,
                        V_sb[:, kt, h, :],
                        PT[:, kt, :],
                        start=(kt == 0),
                        stop=(kt == KT - 1),
                    )
                rs = rc_pool.tile([DH, CH], f32, tag="rs")
                rc = rc_pool.tile([DH, CH], f32, tag="rc")
                nc.vector.tensor_copy(rs, po[DH:P, :])
                nc.vector.reciprocal_approx_fast(rc, rs)
                nc.vector.tensor_mul(
                    OT_sb[p0:p0 + DH, mh, qc, :], po[0:DH, :], rc
                )
                if qc > 0:
                    outproj_t4(qc - 1, h)
        for t4 in range(NCH):
            outproj_t4(NCH - 1, t4)


def _build():
    nc = bacc.Bacc("TRN2", target_bir_lowering=False, debug=False)
    xqT = nc.dram_tensor("xqT", [D, S], bf16, kind="ExternalInput").ap()
    xkT = nc.dram_tensor("xkT", [D, S], bf16, kind="ExternalInput").ap()
    xvT = nc.dram_tensor("xvT", [D, S], bf16, kind="ExternalInput").ap()
    wqT = nc.dram_tensor("wqT", [D, F], bf16, kind="ExternalInput").ap()
    wkT = nc.dram_tensor("wkT", [D, F], bf16, kind="ExternalInput").ap()
    wvT = nc.dram_tensor("wvT", [D, F], bf16, kind="ExternalInput").ap()
    woT = nc.dram_tensor("woT", [F, D], bf16, kind="ExternalInput").ap()
    bq2 = nc.dram_tensor("bq2", [P, MT], f32, kind="ExternalInput").ap()
    bk2 = nc.dram_tensor("bk2", [P, MT], f32, kind="ExternalInput").ap()
    bv1 = nc.dram_tensor("bv1", [F], f32, kind="ExternalInput").ap()
    out = nc.dram_tensor("out", [S, D], bf16, kind="ExternalOutput").ap()
    from contextlib import ExitStack

    with tile.TileContext(nc) as tc, ExitStack() as ctx:
        _emit(ctx, nc, tc,
              (xqT, xkT, xvT, wqT, wkT, wvT, woT, bq2, bk2, bv1, out))
    nc.compile()
    nc.m = get_hw_module(nc.m)
    return nc


_cached_nc = None


def _get_nc():
    global _cached_nc
    if _cached_nc is None:
        _cached_nc = _build()
    return _cached_nc


def make_in_maps(query, key, value, Wq, bq, Wk, bk, Wv, bv, Wo, bo):
    query, key, value, Wq, bq, Wk, bk, Wv, bv, Wo = (
        np.asarray(a, np.float32)
        for a in (query, key, value, Wq, bq, Wk, bk, Wv, bv, Wo)
    )
    xTs = [
        tuple(np.ascontiguousarray(a[b].T).astype(BF16)
              for a in (query, key, value))
        for b in range(B)
    ]
    in_maps = []
    for c in range(N_CORES):
        b, g = divmod(c, 4)
        fs = slice(g * F, (g + 1) * F)
        qT, kT, vT = xTs[b]
        in_maps.append({
            "xqT": qT,
            "xkT": kT,
            "xvT": vT,
            "wqT": np.ascontiguousarray(Wq[fs].T).astype(BF16),
            "wkT": np.ascontiguousarray(Wk[fs].T).astype(BF16),
            "wvT": np.ascontiguousarray(Wv[fs].T).astype(BF16),
            "woT": np.ascontiguousarray(Wo[:, fs].T).astype(BF16),
            "bq2": np.ascontiguousarray(bq[fs].reshape(MT, P).T),
            "bk2": np.ascontiguousarray(bk[fs].reshape(MT, P).T),
            "bv1": np.ascontiguousarray(bv[fs]),
        })
    return in_maps


def combine_outputs(core_outs, bo):
    bo = np.asarray(bo, np.float32)
    out = np.empty((B, S, D), np.float32)
    for b in range(B):
        acc = core_outs[4 * b].astype(np.float32)
        for g in range(1, 4):
            acc = acc + core_outs[4 * b + g].astype(np.float32)
        out[b] = acc + bo
    return out


def kernel(query, key, value, Wq, bq, Wk, bk, Wv, bv, Wo, bo, **run_kwargs):
    nc = _get_nc()
    in_maps = make_in_maps(query, key, value, Wq, bq, Wk, bk, Wv, bv, Wo, bo)
    res = run_bass_kernel_spmd(
        nc, in_maps, core_ids=list(range(N_CORES)), **run_kwargs
    )
    out = combine_outputs([r["out"] for r in res.results], bo)
    if run_kwargs:
        kernel.last_results = res
    return out
